# revision 3
# baseline (speedup 1.0000x reference)
"""Trilinear 3D grid-encoding lookup on 8 TRN2 NeuronCores.

Strategy (data-parallel, per the sharding hint):
  - Host: build a corner-expanded fp16 table E[v, 8*4] where row
    v = (ix*128 + iy)*128 + iz holds the 2x2x2 neighborhood of cell
    (ix,iy,iz) (clamped at the upper edges).  One 64B row per query
    point -> ONE indirect-DMA descriptor per point instead of 8.
  - Shard the (padded) 2M points across 8 cores, 128 partitions x 2048
    points each.
  - Device: per tile of 128x256 points: compute cell index + fractional
    weights with ACT/DVE ops, gather rows with gpsimd indirect DMA,
    blend the 8 corners in fp16 on DVE, emit fp32.
"""
import numpy as np

import concourse.bacc as bacc
import concourse.bass as bass
import concourse.mybir as mybir
from concourse.bass_utils import run_bass_kernel_spmd
from concourse.tile import TileContext

NBINS = 128
OUTC = 4
CORES = 8
P = 128
TPP = 2048                    # points per partition per core
T = 256                       # tile: points per partition
NT = TPP // T
PTS_PER_CORE = P * TPP        # 262144
NPAD = CORES * PTS_PER_CORE   # 2097152
V = NBINS ** 3

F32 = mybir.dt.float32
F16 = mybir.dt.float16
I32 = mybir.dt.int32
AF = mybir.ActivationFunctionType
OP = mybir.AluOpType

# Exposed for the test harness: the BassKernelResults of the last run.
LAST_RESULT = None


def _build():
    nc = bacc.Bacc(None, target_bir_lowering=False)
    xs = nc.dram_tensor("xs", [P, TPP], F32, kind="ExternalInput")
    ys = nc.dram_tensor("ys", [P, TPP], F32, kind="ExternalInput")
    zs = nc.dram_tensor("zs", [P, TPP], F32, kind="ExternalInput")
    tab = nc.dram_tensor("tab", [V, 32], F16, kind="ExternalInput")
    out = nc.dram_tensor("out", [P, TPP, OUTC], F32, kind="ExternalOutput")

    with TileContext(nc) as tc:
        with tc.tile_pool(name="coords", bufs=1) as cpool, \
             tc.tile_pool(name="work", bufs=2) as pool, \
             tc.tile_pool(name="gbuf", bufs=2) as gpool:
            ct = []
            for name, src in (("xt", xs), ("yt", ys), ("zt", zs)):
                t = cpool.tile([P, TPP], F32, name=name)
                nc.sync.dma_start(out=t[:], in_=src[:])
                ct.append(t)

            for it in range(NT):
                sl = bass.ts(it, T)
                fa = []   # fractional parts (f32)
                ia = []   # integer parts as f32
                for ax in range(3):
                    # HW fp32->int32 cast is round-to-nearest-even, so
                    # floor(p) == rne(p - 0.5) (integer ties land one cell
                    # lower with f == 1.0 -- the same lerp, still in
                    # bounds thanks to the corner-expanded table).
                    ph_a = pool.tile([P, T], F32, name=f"ph{ax}")
                    nc.scalar.activation(out=ph_a[:], in_=ct[ax][:, sl],
                                         func=AF.Copy, scale=float(NBINS),
                                         bias=-0.5)
                    ii_a = pool.tile([P, T], I32, name=f"ii{ax}")
                    nc.vector.tensor_copy(out=ii_a[:], in_=ph_a[:])
                    i_a = pool.tile([P, T], F32, name=f"i{ax}")
                    nc.vector.tensor_copy(out=i_a[:], in_=ii_a[:])
                    f_a = pool.tile([P, T], F32, name=f"f{ax}")
                    nc.vector.scalar_tensor_tensor(out=f_a[:], in0=ph_a[:],
                                                   scalar=0.5, in1=i_a[:],
                                                   op0=OP.add, op1=OP.subtract)
                    fa.append(f_a)
                    ia.append(i_a)

                # idx = (ix*128 + iy)*128 + iz   (exact in f32), then int32
                t1 = pool.tile([P, T], F32, name="t1")
                nc.vector.scalar_tensor_tensor(out=t1[:], in0=ia[0][:],
                                               scalar=float(NBINS), in1=ia[1][:],
                                               op0=OP.mult, op1=OP.add)
                t2 = pool.tile([P, T], F32, name="t2")
                nc.vector.scalar_tensor_tensor(out=t2[:], in0=t1[:],
                                               scalar=float(NBINS), in1=ia[2][:],
                                               op0=OP.mult, op1=OP.add)
                idx = pool.tile([P, T], I32, name="idx")
                nc.vector.tensor_copy(out=idx[:], in_=t2[:])

                # gather the 2x2x2 neighborhoods: one 64B row per point.
                # The vector-indirect (multi-index) DMA form miscompiles in
                # this toolchain; only one index per partition per
                # instruction works, so issue T column gathers.
                G = gpool.tile([P, T, 32], F16, name="G")
                for j in range(T):
                    nc.gpsimd.indirect_dma_start(
                        out=G[:, j, :], out_offset=None,
                        in_=tab[:],
                        in_offset=bass.IndirectOffsetOnAxis(
                            ap=idx[:, j:j + 1], axis=0),
                    )

                # complements (1 - f) on ACT
                ca = []
                for ax in range(3):
                    c_a = pool.tile([P, T], F32, name=f"c{ax}")
                    nc.scalar.activation(out=c_a[:], in_=fa[ax][:],
                                         func=AF.Copy, scale=-1.0, bias=1.0)
                    ca.append(c_a)

                # corner weights w8[k], k = dx*4 + dy*2 + dz  (f32 -> f16)
                wxy = []
                for dx in range(2):
                    for dy in range(2):
                        w = pool.tile([P, T], F32, name=f"wxy{dx}{dy}")
                        nc.vector.tensor_tensor(
                            out=w[:],
                            in0=(fa[0] if dx else ca[0])[:],
                            in1=(fa[1] if dy else ca[1])[:],
                            op=OP.mult)
                        wxy.append(w)
                w16 = []
                for k in range(8):
                    dz = k & 1
                    w = pool.tile([P, T], F32, name=f"w8_{k}")
                    nc.vector.tensor_tensor(
                        out=w[:],
                        in0=wxy[k >> 1][:],
                        in1=(fa[2] if dz else ca[2])[:],
                        op=OP.mult)
                    w16_k = pool.tile([P, T], F16, name=f"w16_{k}")
                    nc.scalar.activation(out=w16_k[:], in_=w[:], func=AF.Copy)
                    w16.append(w16_k)

                # blend: acc[p,t,c] = sum_k w16[k] * G[:, :, 4k:4k+4]
                acc = pool.tile([P, T, OUTC], F16, name="acc")
                prod = pool.tile([P, T, OUTC], F16, name="prod")
                for k in range(8):
                    tgt = acc if k == 0 else prod
                    nc.vector.tensor_tensor(
                        out=tgt[:],
                        in0=G[:, :, 4 * k:4 * k + 4],
                        in1=w16[k][:].unsqueeze(-1).to_broadcast([P, T, OUTC]),
                        op=OP.mult)
                    if k > 0:
                        nc.vector.tensor_tensor(out=acc[:], in0=acc[:],
                                                in1=prod[:], op=OP.add)
                acc32 = pool.tile([P, T, OUTC], F32, name="acc32")
                nc.scalar.activation(out=acc32[:], in_=acc[:], func=AF.Copy)
                nc.sync.dma_start(out=out[:, sl, :], in_=acc32[:])
    nc.compile()
    return nc


def _build_table(grid: np.ndarray) -> np.ndarray:
    g = np.asarray(grid, dtype=np.float32)
    gp = np.pad(g, ((0, 1), (0, 1), (0, 1), (0, 0)), mode="edge")
    w = np.lib.stride_tricks.sliding_window_view(gp, (2, 2, 2), axis=(0, 1, 2))
    # w: [128,128,128, 4, 2,2,2] with trailing (dx,dy,dz); want (dx,dy,dz,c)
    e = w.transpose(0, 1, 2, 4, 5, 6, 3).reshape(V, 32)
    return np.ascontiguousarray(e, dtype=np.float16)


def kernel(x: np.ndarray, grid: np.ndarray) -> np.ndarray:
    global LAST_RESULT
    x = np.asarray(x, dtype=np.float32)
    n = x.shape[0]
    tab = _build_table(grid)

    xp = np.zeros((NPAD, 3), dtype=np.float32)
    xp[:n] = x
    xp = xp.reshape(CORES, P, TPP, 3)

    in_maps = []
    for c in range(CORES):
        in_maps.append({
            "xs": np.ascontiguousarray(xp[c, :, :, 0]),
            "ys": np.ascontiguousarray(xp[c, :, :, 1]),
            "zs": np.ascontiguousarray(xp[c, :, :, 2]),
            "tab": tab,
        })

    nc = _build()
    res = run_bass_kernel_spmd(nc, in_maps, core_ids=list(range(CORES)))
    LAST_RESULT = res
    out = np.concatenate(
        [np.asarray(r["out"], dtype=np.float32).reshape(PTS_PER_CORE, OUTC)
         for r in res.results], axis=0)
    return out[:n]
